# revision 1
# baseline (speedup 1.0000x reference)
"""Single-head GAT (DGL GATConv) forward on 8 Trainium2 NeuronCores.

Strategy (graph/data parallel, per the sharding hint):
  - Nodes padded 10000 -> 10240, sharded 1280/core (8 cores).
  - Phase 1 (per core): h_shard = feats_shard @ [W | W@attn_l] in one PE
    matmul pair (el appended as an extra output column); er for the shard
    is produced directly in ROW layout via lhsT = (W@attn_r).  Augmented
    rows Haug[n] = [h+bias bf16 (256) | 1.0 | el_hi | el_lo] (260 bf16 =
    520 B; bias is folded in so out = u'/s needs no separate bias add)
    staged to DRAM; AllGather -> full Haug [10240, 260].
  - Phase 2 (per core): nodes are degree-balance relabeled on the host
    (greedy bin packing) so each of the 10 windows of 128 dst nodes per
    core holds <= EPW = 4096 in-edges (32 chunks); edges are dst-sorted
    into those windows.  Per window:
      * 32x indirect_dma_start gathers (128 rows each) of Haug rows by src
        (the purpose-built dma_gather/dma_scatter_add ucode instructions
        crash this runtime; multi-row indirect DMA is also broken, so
        one-row-per-partition indirect DMA is the only working gather)
      * Sel[e,c,slot] = (iota == slot_e) built once per window in ONE
        broadcast-AP DVE op; er per edge = reduce_X(Sel * er_rep) where
        er_rep is er broadcast to all partitions via a K=1 matmul -- no
        er gather at all, er stays exact fp32
      * w = exp(leakyrelu(el + er)) batched over the window (DVE+ACT);
        SelW = Sel * w in one broadcast op
      * 32 PE matmuls accumulate psum[128 slots, 257] += SelW^T @ [h | 1]:
        unnormalized sums u and softmax denominator s in one pass (no
        max-subtraction needed: |logit| <= ~12)
      * epilogue: out = u / s + bias -> DMA to the output shard.  (s == 0
        only on padding slots whose rows the host discards.)

The environment executes ~40us/instruction (measured), so the kernel is
optimized for minimum instruction count, not bandwidth.

kernel(**inputs) takes full unsharded inputs, returns [10000, 256] fp32.
"""

import numpy as np
import ml_dtypes

N = 10000
E = 320000
D = 256
NPAD = 10240
NCORES = 8
SH = NPAD // NCORES          # 1280 nodes per core
WINN = 128                   # dst nodes per window
NW = SH // WINN              # 10 windows per core
EPW = 4096                   # padded edges per window (multiple of 128)
CH = EPW // 128              # 32 chunks of 128 edges per window
DA = 260                     # bf16 elements per augmented row (520 B)
NEG_SLOPE = 0.2
REPEAT = 1                   # whole-pipeline repeats (for differential timing)
ABLATE_GATHER = False        # timing ablation: skip indirect gathers
ABLATE_COLL = False          # timing ablation: skip the AllGather

_BF16 = ml_dtypes.bfloat16

_prog_cache = {}


def _prep_inputs(feats, W, attn_l, attn_r, bias, src, dst):
    """Host-side sharding/index prep. Returns in_maps (one dict per core)."""
    feats_pad = np.zeros((NPAD, D), np.float32)
    feats_pad[:N] = feats
    # fold attn vectors through W so el/er come straight from feats
    val = (W @ attn_l).astype(np.float32)
    var = (W @ attn_r).astype(np.float32)
    w_ext = np.concatenate([W.astype(np.float32), val[:, None]], axis=1)  # [256,257]

    # degree-balanced node relabeling: greedy-pack nodes into 80 windows of
    # 128 so every window has <= EPW in-edges (real data: max 4001)
    import heapq
    nwin_g = NPAD // WINN                     # 80
    deg = np.bincount(dst, minlength=NPAD).astype(np.int64)
    norder = np.argsort(-deg, kind="stable")
    bin_edges = np.zeros(nwin_g, np.int64)
    bin_count = np.zeros(nwin_g, np.int64)
    newid = np.empty(NPAD, np.int64)
    heap = [(0, b) for b in range(nwin_g)]
    heapq.heapify(heap)
    for n in norder:
        while True:
            _, b = heapq.heappop(heap)
            if bin_count[b] < WINN:
                break
        newid[n] = b * WINN + bin_count[b]
        bin_count[b] += 1
        bin_edges[b] += deg[n]
        if bin_count[b] < WINN:
            heapq.heappush(heap, (bin_edges[b], b))
    assert bin_edges.max() <= EPW, (bin_edges.max(), EPW)
    inv = np.empty(NPAD, np.int64)
    inv[newid] = np.arange(NPAD)
    feats_pad = feats_pad[inv]                # row j of feats_pad = new id j

    n_src = newid[src.astype(np.int64)]
    n_dst = newid[dst.astype(np.int64)]
    order = np.argsort(n_dst, kind="stable")
    s_src = n_src[order]
    s_dst = n_dst[order]

    win = s_dst // WINN                       # global window id, 0..79
    counts = np.bincount(win, minlength=nwin_g)
    starts = np.concatenate([[0], np.cumsum(counts)])

    src_idx = np.zeros((nwin_g, EPW), np.int32)
    slot = np.full((nwin_g, EPW), -1.0, np.float32)   # dst - window_base, -1 pad
    for g in range(nwin_g):
        a, b = starts[g], starts[g + 1]
        k = b - a
        src_idx[g, :k] = s_src[a:b]
        slot[g, :k] = s_dst[a:b] - g * WINN

    # per-chunk per-partition layout: token i -> [i % 128, i // 128]
    def tok(x):  # [nw, EPW] -> [nw, 128, CH]
        return np.ascontiguousarray(x.reshape(x.shape[0], CH, 128).transpose(0, 2, 1))

    comb = np.empty((nwin_g, 128, 2, CH), np.int32)
    comb[:, :, 0, :] = tok(src_idx)
    comb[:, :, 1, :] = tok(slot).view(np.int32)       # f32 bits in i32 array

    iota_row = np.broadcast_to(np.arange(128, dtype=np.float32), (128, 128))
    iota_row = np.ascontiguousarray(iota_row).astype(_BF16)

    in_maps = []
    for c in range(NCORES):
        featsT = np.ascontiguousarray(feats_pad[c * SH:(c + 1) * SH].T)  # [256, SH]
        lo, hi = c * NW, (c + 1) * NW
        in_maps.append({
            "featsT": featsT,
            "Wext": w_ext,
            "var_in": np.ascontiguousarray(var[:, None]),
            "bias_in": np.ascontiguousarray(bias.astype(np.float32)[None, :]),
            "iota_row": iota_row,
            "comb": np.ascontiguousarray(comb[lo:hi]),
        })
    return in_maps, newid


def _build_program(ncores):
    import concourse.bass as bass
    import concourse.tile as tile
    from concourse import bacc, mybir
    from contextlib import ExitStack

    f32 = mybir.dt.float32
    bf16 = mybir.dt.bfloat16
    i32 = mybir.dt.int32

    nc = bacc.Bacc(
        "TRN2", target_bir_lowering=False, debug=False, num_devices=ncores
    )

    featsT = nc.dram_tensor("featsT", [D, SH], f32, kind="ExternalInput").ap()
    Wext = nc.dram_tensor("Wext", [D, D + 1], f32, kind="ExternalInput").ap()
    var_in = nc.dram_tensor("var_in", [D, 1], f32, kind="ExternalInput").ap()
    bias_in = nc.dram_tensor("bias_in", [1, D], f32, kind="ExternalInput").ap()
    iota_in = nc.dram_tensor("iota_row", [128, 128], bf16, kind="ExternalInput").ap()
    combw = nc.dram_tensor("comb", [NW, 128, 2, CH], i32, kind="ExternalInput").ap()
    out_ext = nc.dram_tensor("out", [SH, D], f32, kind="ExternalOutput").ap()

    hstage = nc.dram_tensor("hstage", [SH, DA], bf16).ap()
    if ncores > 1:
        hfull = nc.dram_tensor("hfull", [NPAD, DA], bf16, addr_space="Shared").ap()
    else:
        hfull = hstage

    NT = SH // 128  # node tiles per core

    with tile.TileContext(nc) as tc, ExitStack() as ctx:
        const = ctx.enter_context(tc.tile_pool(name="const", bufs=1))

        w_sb = const.tile([128, 2, D + 1], f32, tag="w_sb")   # Wext (2 k-halves)
        nc.sync.dma_start(w_sb[:, 0, :], Wext[0:128, :])
        nc.sync.dma_start(w_sb[:, 1, :], Wext[128:256, :])
        var_sb = const.tile([128, 2, 1], f32, tag="var_sb")
        nc.sync.dma_start(var_sb[:, 0, :], var_in[0:128, :])
        nc.sync.dma_start(var_sb[:, 1, :], var_in[128:256, :])
        iota_sb = const.tile([128, 128], bf16, tag="iota")
        nc.sync.dma_start(iota_sb[:], iota_in[:])
        bias_row = const.tile([1, D], f32, tag="bias_row")
        nc.sync.dma_start(bias_row[:], bias_in[:])
        ones_col = const.tile([1, 128], f32, tag="ones_col")
        nc.vector.memset(ones_col[:], 1.0)
        er_rows = const.tile([1, SH], f32, tag="er_rows")

        bias_rep = const.tile([128, D], f32, tag="bias_rep")
        with tc.tile_pool(name="psum_b", bufs=1, space="PSUM") as psb:
            pb = psb.tile([128, D], f32)
            nc.tensor.matmul(pb[:], lhsT=ones_col[:], rhs=bias_row[:],
                             start=True, stop=True)
            nc.vector.tensor_copy(bias_rep[:], pb[:])

        for _rep in range(REPEAT):
            # ------------- Phase 1: h, el, er for the local shard ----------
            with tc.tile_pool(name="p1_sbuf", bufs=2) as p1, \
                 tc.tile_pool(name="p1_big", bufs=1) as p1b, \
                 tc.tile_pool(name="p1_psum", bufs=2, space="PSUM") as pp:
                ftT = p1b.tile([128, 2, SH], f32, tag="ftT")
                nc.sync.dma_start(ftT[:, 0, :], featsT[0:128, :])
                nc.sync.dma_start(ftT[:, 1, :], featsT[128:256, :])

                hbig = p1b.tile([128, NT, DA], bf16, tag="hbig")
                nc.vector.memset(hbig[:, :, D:D + 1], 1.0)
                nc.vector.memset(hbig[:, :, 259:260], 0.0)
                for j in range((SH + 511) // 512):
                    lo, hi = j * 512, min((j + 1) * 512, SH)
                    erp = pp.tile([128, 512], f32, tag="erp")
                    for k in range(2):
                        nc.tensor.matmul(erp[0:1, 0:hi - lo], lhsT=var_sb[:, k, :],
                                         rhs=ftT[:, k, lo:hi],
                                         start=(k == 0), stop=(k == 1),
                                         tile_position=(0, 0))
                    nc.vector.tensor_copy(er_rows[:, lo:hi], erp[0:1, 0:hi - lo])
                for nt in range(NT):
                    nsl = bass.ts(nt, 128)
                    hp = pp.tile([128, D + 1], f32, tag="hp")
                    for k in range(2):
                        nc.tensor.matmul(hp[:], lhsT=ftT[:, k, nsl],
                                         rhs=w_sb[:, k, :],
                                         start=(k == 0), stop=(k == 1))
                    # store h + bias (so out = u'/s needs no separate bias add)
                    nc.vector.tensor_tensor(out=hbig[:, nt, 0:D], in0=hp[:, 0:D],
                                            in1=bias_rep[:],
                                            op=mybir.AluOpType.add)
                    # el as hi/lo bf16 pair (cols 257, 258): el ~= hi + lo
                    nc.vector.tensor_copy(hbig[:, nt, 257:258], hp[:, D:D + 1])
                    lo32 = p1.tile([128, 1], f32, tag="lo32")
                    nc.vector.tensor_tensor(out=lo32[:], in0=hp[:, D:D + 1],
                                            in1=hbig[:, nt, 257:258],
                                            op=mybir.AluOpType.subtract)
                    nc.vector.tensor_copy(hbig[:, nt, 258:259], lo32[:])
                # single staging DMA: hbig [p, nt, :] -> hstage [nt*128+p, :]
                nc.sync.dma_start(
                    hstage.rearrange("(t p) d -> p t d", p=128), hbig[:])

            if ncores > 1 and not ABLATE_COLL:
                nc.gpsimd.collective_compute(
                    "AllGather",
                    mybir.AluOpType.bypass,
                    replica_groups=[list(range(ncores))],
                    ins=[hstage[:]],
                    outs=[hfull[:]],
                )

            # ------------- Phase 2: per-window attention + aggregation -----
            with tc.tile_pool(name="gidx", bufs=2) as gidx_p, \
                 tc.tile_pool(name="ghr", bufs=3) as ghr_p, \
                 tc.tile_pool(name="sel", bufs=3) as sel_p, \
                 tc.tile_pool(name="erpr", bufs=1) as erpr_p, \
                 tc.tile_pool(name="erprod", bufs=2) as erprod_p, \
                 tc.tile_pool(name="small", bufs=2) as small_p, \
                 tc.tile_pool(name="selw", bufs=2) as selw_p, \
                 tc.tile_pool(name="outp", bufs=2) as out_p, \
                 tc.tile_pool(name="psum_u", bufs=2, space="PSUM") as psu_p, \
                 tc.tile_pool(name="psum_r", bufs=2, space="PSUM") as psr_p:
                # er replicated to all partitions for ALL windows (K=1 matmuls)
                er_rep = erpr_p.tile([128, NW, 128], f32, tag="er_rep")
                for j in range(NW // 2):
                    err_ps = psr_p.tile([128, 2, 128], f32, tag="err_ps")
                    nc.tensor.matmul(err_ps[:], lhsT=ones_col[:],
                                     rhs=er_rows[:, j * 256:(j + 1) * 256],
                                     start=True, stop=True)
                    nc.vector.tensor_copy(er_rep[:, 2 * j:2 * j + 2, :], err_ps[:])
                comb = gidx_p.tile([128, NW, 2, CH], i32, tag="comb")
                nc.sync.dma_start(comb[:], combw.transpose([1, 0, 2, 3]))
                for w in range(NW):
                    six = comb[:, w, 0, :]
                    slot_f = comb[:, w, 1, :].bitcast(f32)

                    ghr = ghr_p.tile([128, CH, DA], bf16, tag="ghr")
                    if ABLATE_GATHER:
                        nc.vector.memset(ghr[:], 0.5)
                    else:
                        for c in range(CH):
                            nc.gpsimd.indirect_dma_start(
                                out=ghr[:, c, :], out_offset=None, in_=hfull[:],
                                in_offset=bass.IndirectOffsetOnAxis(
                                    ap=six[:, c:c + 1], axis=0))

                    # Sel for ALL chunks in one broadcast-AP op
                    sel = sel_p.tile([128, CH, 128], bf16, tag="sel")
                    nc.vector.tensor_tensor(
                        out=sel[:],
                        in0=iota_sb[:, None, :].broadcast_to([128, CH, 128]),
                        in1=slot_f[:, :, None].broadcast_to([128, CH, 128]),
                        op=mybir.AluOpType.is_equal,
                    )
                    # er per edge = reduce_X(Sel * er_rep)  (exact fp32)
                    er_prod = erprod_p.tile([128, CH, 130], f32, tag="er_prod")
                    nc.vector.tensor_tensor(
                        out=er_prod[:, :, 0:128],
                        in0=sel[:],
                        in1=er_rep[:, w, None, :].broadcast_to([128, CH, 128]),
                        op=mybir.AluOpType.mult,
                    )
                    # el hi/lo pair goes into the same reduction
                    nc.vector.tensor_copy(er_prod[:, :, 128:130],
                                          ghr[:, :, 257:259])
                    t_sb = small_p.tile([128, CH], f32, tag="t")
                    nc.vector.tensor_reduce(
                        out=t_sb[:], in_=er_prod[:],
                        axis=mybir.AxisListType.X, op=mybir.AluOpType.add,
                    )
                    t2_sb = small_p.tile([128, CH], f32, tag="t2")
                    nc.vector.tensor_scalar_mul(t2_sb[:], t_sb[:], NEG_SLOPE)
                    lk_sb = small_p.tile([128, CH], f32, tag="lk")
                    nc.vector.tensor_tensor(
                        out=lk_sb[:], in0=t_sb[:], in1=t2_sb[:],
                        op=mybir.AluOpType.max,
                    )
                    wv_sb = small_p.tile([128, CH], f32, tag="wv")
                    nc.scalar.activation(
                        wv_sb[:], lk_sb[:], mybir.ActivationFunctionType.Exp
                    )

                    selw = selw_p.tile([128, CH, 128], bf16, tag="selw")
                    nc.vector.tensor_tensor(
                        out=selw[:],
                        in0=sel[:],
                        in1=wv_sb[:, :, None].broadcast_to([128, CH, 128]),
                        op=mybir.AluOpType.mult,
                    )
                    psum_u = psu_p.tile([128, D + 1], f32, tag="pu")
                    for c in range(CH):
                        nc.tensor.matmul(
                            psum_u[:], lhsT=selw[:, c, :], rhs=ghr[:, c, 0:D + 1],
                            start=(c == 0), stop=(c == CH - 1),
                        )

                    # out = u / s + bias  (s==0 only on discarded pad slots)
                    rcp = small_p.tile([128, 1], f32, tag="rcp")
                    nc.vector.reciprocal(rcp[:], psum_u[:, D:D + 1])
                    ot = out_p.tile([128, D], f32, tag="ot")
                    nc.vector.tensor_scalar_mul(ot[:], psum_u[:, 0:D], rcp[:])
                    nc.sync.dma_start(out_ext[w * 128:(w + 1) * 128, :], ot[:])

    nc.compile()
    return nc


def _get_program(ncores):
    if ncores not in _prog_cache:
        _prog_cache[ncores] = _build_program(ncores)
    return _prog_cache[ncores]


def kernel(feats, W, attn_l, attn_r, bias, src, dst):
    from concourse.bass_utils import run_bass_kernel_spmd

    feats = np.asarray(feats, np.float32)
    W = np.asarray(W, np.float32)
    attn_l = np.asarray(attn_l, np.float32)
    attn_r = np.asarray(attn_r, np.float32)
    bias = np.asarray(bias, np.float32)
    src = np.asarray(src)
    dst = np.asarray(dst)

    in_maps, newid = _prep_inputs(feats, W, attn_l, attn_r, bias, src, dst)
    nc = _get_program(NCORES)
    res = run_bass_kernel_spmd(nc, in_maps, list(range(NCORES)))
    shards = [np.asarray(res.results[c]["out"]) for c in range(NCORES)]
    out_cat = np.concatenate(shards, axis=0)
    return out_cat[newid[:N]].astype(np.float32)



# revision 7
# speedup vs baseline: 5.3641x; 5.3641x over previous
"""Single-head GAT (DGL GATConv) forward on 8 Trainium2 NeuronCores.

Strategy (graph/data parallel, per the sharding hint), v2 — prefix-sum
aggregation. This environment executes instructions at a large flat cost
(~60-200us each, engine-dependent: DVE ~60us, Act ~110us, PE matmul
~190us, ap_gather [128,4096,4] ~120us, AllGather ~1ms), so the kernel is
shaped to minimize INSTRUCTION COUNT, using the largest possible access
patterns per instruction.

  - Nodes padded 10000 -> 10240, degree-balance relabeled into 80 windows
    of 128 dst nodes such that each window has <= 4096 in-edges; 10
    windows per core (1280 dst nodes/core); edges dst-sorted per window.
  - Phase 1 (per core): hT = W^T @ feats^T computed FEATURE-MAJOR: the
    local shard's table row n holds bf16 (h[n,p], h[n,p+128], el_hi,
    el_lo) on partition p (el split hi/lo bf16 for ~f32 accuracy);
    el/er rows from a [256,2] (W@attn_l | W@attn_r) matmul,
    partition-replicated via the gpsimd partition_broadcast ucode op.
    Staged to DRAM, AllGather -> full table [128, 10240, 4].
  - Phase 2 (per core, per window): ONE ap_gather pulls all 4096 edges'
    (h-pair, el-pair) columns; a second ap_gather pulls per-edge er (f32,
    by local dst). Logits e = lrelu(el+er) batched on [128,4096]; a
    per-window max is subtracted before exp (softmax shift-invariance per
    dst; all of a dst's edges live in one window, so it cancels exactly)
    which keeps the prefix sums O(1)-magnitude. wh[e] = w_e * h-pair and
    w_e are packed into a [128, 3, 4+4096] bf16 buffer (4 leading zeros
    per channel); ONE tensor_tensor_scan computes the running sum of all
    3 channels; ONE ap_gather reads the 3*129 segment-boundary prefix
    values; ONE subtract turns them into per-dst (u0, u1, s) =
    (sum w*h_p, sum w*h_{p+128}, sum w). Segment sums via prefix
    differences are exact in structure (the scan state is fp32).
  - Epilogue: out^T = u/s + bias in 3 DVE ops + 1 DMA; host de-transposes
    and un-relabels.

kernel(**inputs) takes full unsharded inputs, returns [10000, 256] fp32.
"""

import numpy as np
import ml_dtypes

N = 10000
E = 320000
D = 256
NPAD = 10240
NCORES = 8
SH = NPAD // NCORES          # 1280 nodes per core
WINN = 128                   # dst nodes per window
NW = SH // WINN              # 10 windows per core
EPW = 4096                   # padded edges per window
CW = 4100                    # scan channel width: 4 zero pads + 4096
NEG_SLOPE = 0.2
REPEAT = 1                   # whole-pipeline repeats (differential timing)

_BF16 = ml_dtypes.bfloat16

_prog_cache = {}


def _wrap16(arr):
    """[Q] -> [128, Q//16] int16 in ap_gather's wrapped-index layout:
    idx[16g + r, f] = arr[f*16 + r], replicated across the 8 groups g."""
    q = arr.shape[0]
    w16 = arr.reshape(q // 16, 16).T.astype(np.int16)     # [16, Q//16]
    return np.broadcast_to(w16[None], (8, 16, q // 16)).reshape(128, q // 16)


def _prep_inputs(feats, W, attn_l, attn_r, bias, src, dst):
    """Host-side sharding/index prep. Returns (in_maps, newid)."""
    feats_pad = np.zeros((NPAD, D), np.float32)
    feats_pad[:N] = feats
    val = (W @ attn_l).astype(np.float32)
    var = (W @ attn_r).astype(np.float32)

    # degree-balanced node relabeling: greedy-pack nodes into 80 windows of
    # 128 so every window has <= EPW in-edges
    import heapq
    nwin_g = NPAD // WINN                     # 80
    deg = np.bincount(dst, minlength=NPAD).astype(np.int64)
    norder = np.argsort(-deg, kind="stable")
    bin_edges = np.zeros(nwin_g, np.int64)
    bin_count = np.zeros(nwin_g, np.int64)
    newid = np.empty(NPAD, np.int64)
    heap = [(0, b) for b in range(nwin_g)]
    heapq.heapify(heap)
    for n in norder:
        while True:
            _, b = heapq.heappop(heap)
            if bin_count[b] < WINN:
                break
        newid[n] = b * WINN + bin_count[b]
        bin_count[b] += 1
        bin_edges[b] += deg[n]
        if bin_count[b] < WINN:
            heapq.heappush(heap, (bin_edges[b], b))
    assert bin_edges.max() <= EPW, (bin_edges.max(), EPW)
    inv = np.empty(NPAD, np.int64)
    inv[newid] = np.arange(NPAD)
    feats_pad = feats_pad[inv]                # row j of feats_pad = new id j

    n_src = newid[src.astype(np.int64)]
    n_dst = newid[dst.astype(np.int64)]
    order = np.argsort(n_dst, kind="stable")
    s_src = n_src[order]
    s_dst = n_dst[order]

    win = s_dst // WINN                       # global window id, 0..79
    counts = np.bincount(win, minlength=nwin_g)
    starts = np.concatenate([[0], np.cumsum(counts)])

    # per-window index tables: h-gather (global src), er-gather (local dst),
    # boundary-gather (3 channels x 129 prefix positions in the scan buffer)
    # 544 (not 537): keeps every idx slice 64-byte aligned in all
    # windows; odd-element offsets make the gpsimd idx reads garbage
    comb = np.zeros((nwin_g, 128, 544), np.int16)
    for g in range(nwin_g):
        a, b = starts[g], starts[g + 1]
        k = b - a
        hidx = np.zeros(EPW, np.int64)
        eidx = np.zeros(EPW, np.int64)
        hidx[:k] = s_src[a:b]
        core_base = (g // NW) * SH
        eidx[:k] = s_dst[a:b] - core_base
        slot = s_dst[a:b] - g * WINN          # 0..127, ascending
        cum = np.zeros(129, np.int64)         # cum[j] = #edges with slot < j
        cnt = np.bincount(slot, minlength=128)
        cum[1:] = np.cumsum(cnt)
        bidx = np.zeros(400, np.int64)
        for ch in range(3):
            bidx[132 * ch:132 * ch + 129] = CW * ch + 3 + cum
        comb[g, :, 0:256] = _wrap16(hidx)
        comb[g, :, 256:512] = _wrap16(eidx)
        comb[g, :, 512:537] = _wrap16(bidx)

    # weight tiles: Wt4[kp, kh, ft, fp] = W[kp+128kh, fp+128ft]
    w4 = W.astype(np.float32).reshape(2, 128, 2, 128)          # [kh,kp,ft,fp]
    Wt4 = np.ascontiguousarray(w4.transpose(1, 0, 2, 3))       # [kp,kh,ft,fp]
    valvar = np.stack([val, var], axis=1).reshape(2, 128, 2)   # [kh,kp,2]
    valvar = np.ascontiguousarray(valvar.transpose(1, 0, 2))   # [kp,kh,2]
    bias2 = np.ascontiguousarray(
        bias.astype(np.float32).reshape(2, 128).T)             # [p, d]

    in_maps = []
    for c in range(NCORES):
        fsh = feats_pad[c * SH:(c + 1) * SH]                   # [SH, 256]
        ftT = np.ascontiguousarray(
            fsh.T.reshape(2, 128, SH).transpose(1, 0, 2))      # [kp,kh,n]
        in_maps.append({
            "ftT": ftT,
            "Wt4": Wt4,
            "valvar": valvar,
            "bias2": bias2,
            "comb": np.ascontiguousarray(comb[c * NW:(c + 1) * NW]),
        })
    return in_maps, newid


def _build_program(ncores):
    import concourse.bass as bass
    import concourse.tile as tile
    from concourse import bacc, mybir
    from contextlib import ExitStack

    f32 = mybir.dt.float32
    bf16 = mybir.dt.bfloat16
    i16 = mybir.dt.int16

    nc = bacc.Bacc(
        "TRN2", target_bir_lowering=False, debug=False, num_devices=ncores
    )

    ftT_in = nc.dram_tensor("ftT", [128, 2, SH], f32, kind="ExternalInput").ap()
    Wt4_in = nc.dram_tensor("Wt4", [128, 2, 2, 128], f32, kind="ExternalInput").ap()
    vv_in = nc.dram_tensor("valvar", [128, 2, 2], f32, kind="ExternalInput").ap()
    b2_in = nc.dram_tensor("bias2", [128, 2], f32, kind="ExternalInput").ap()
    comb_in = nc.dram_tensor("comb", [NW, 128, 544], i16, kind="ExternalInput").ap()
    out_ext = nc.dram_tensor("out", [128, 2, SH], f32, kind="ExternalOutput").ap()

    hstage = nc.dram_tensor("hstage", [128, SH * 4], bf16).ap()
    if ncores > 1:
        hfull = nc.dram_tensor("hfull", [ncores, 128, SH * 4], bf16,
                               addr_space="Shared").ap()
    else:
        hfull = hstage

    add = mybir.AluOpType.add
    mult = mybir.AluOpType.mult
    amax = mybir.AluOpType.max
    sub = mybir.AluOpType.subtract

    with tile.TileContext(nc) as tc, ExitStack() as ctx:
        const = ctx.enter_context(tc.tile_pool(name="const", bufs=1))
        wt = const.tile([128, 2, 2, 128], f32, tag="wt")
        nc.sync.dma_start(wt[:], Wt4_in[:])
        vv = const.tile([128, 2, 2], f32, tag="vv")
        nc.sync.dma_start(vv[:], vv_in[:])
        b2 = const.tile([128, 2], f32, tag="b2")
        nc.sync.dma_start(b2[:], b2_in[:])
        combs = const.tile([128, NW, 544], i16, tag="combs")
        nc.sync.dma_start(combs[:], comb_in.transpose([1, 0, 2]))
        zrow = const.tile([128, 1], f32, tag="zrow")
        nc.vector.memset(zrow[:], 0.0)
        usall = const.tile([128, 3, SH], f32, tag="usall")
        er_rep = const.tile([128, SH], f32, tag="er_rep")

        for _rep in range(REPEAT):
            # ---- Phase 1: feature-major h + el/er for the local shard ----
            with tc.tile_pool(name="p1", bufs=1) as p1:
                ftT = p1.tile([128, 2, SH], f32, tag="ftT")
                nc.sync.dma_start(ftT[:, 0, :], ftT_in[:, 0, :])
                nc.sync.dma_start(ftT[:, 1, :], ftT_in[:, 1, :])

                bounds = [(0, 512), (512, 1024), (1024, 1280)]
                els = p1.tile([2, SH], f32, tag="els")
                with tc.tile_pool(name="p1pse", bufs=1, space="PSUM") as ppe:
                    pse = ppe.tile([2, 1536], f32, tag="pse")
                    for kh in range(2):
                        for c0, c1 in bounds:
                            nc.tensor.matmul(
                                pse[:, c0:c1], lhsT=vv[:, kh, :],
                                rhs=ftT[:, kh, c0:c1],
                                start=(kh == 0), stop=(kh == 1))
                    nc.vector.tensor_copy(els[:], pse[:, 0:SH])

                hst = p1.tile([128, SH, 4], bf16, tag="hst")
                with tc.tile_pool(name="p1psh", bufs=1, space="PSUM") as pph:
                    psh = pph.tile([128, 2, 1536], f32, tag="psh")
                    for ft in range(2):
                        for kh in range(2):
                            for c0, c1 in bounds:
                                nc.tensor.matmul(
                                    psh[:, ft, c0:c1], lhsT=wt[:, kh, ft, :],
                                    rhs=ftT[:, kh, c0:c1],
                                    start=(kh == 0), stop=(kh == 1))
                    nc.vector.tensor_copy(hst[:, :, 0], psh[:, 0, 0:SH])
                    nc.vector.tensor_copy(hst[:, :, 1], psh[:, 1, 0:SH])
                el_rep = p1.tile([128, SH], f32, tag="el_rep")
                ers = p1.tile([1, SH], f32, tag="ers")
                nc.sync.dma_start(ers[:], els[1:2, :])
                nc.gpsimd.partition_broadcast(el_rep[:], els[0:1, :],
                                              channels=128)
                nc.gpsimd.partition_broadcast(er_rep[:], ers[0:1, :],
                                              channels=128)
                nc.vector.tensor_copy(hst[:, :, 2], el_rep[:])
                nc.vector.tensor_tensor(out=hst[:, :, 3], in0=el_rep[:],
                                        in1=hst[:, :, 2], op=sub)
                nc.sync.dma_start(
                    hstage[:], hst[:].rearrange("p n d -> p (n d)"))

            if ncores > 1:
                nc.gpsimd.collective_compute(
                    "AllGather", mybir.AluOpType.bypass,
                    replica_groups=[list(range(ncores))],
                    ins=[hstage[:]], outs=[hfull[:]],
                )

            # ---- Phase 2: per-window gather + softmax + prefix-sum agg ----
            with tc.tile_pool(name="p2", bufs=1) as p2, \
                 tc.tile_pool(name="p2ps", bufs=1, space="PSUM") as pp2:
                hTi = p2.tile([128, ncores * SH * 4], bf16, tag="hTi")
                if ncores > 1:
                    nc.sync.dma_start(hTi[:], hfull.transpose([1, 0, 2]))
                else:
                    nc.sync.dma_start(hTi[:], hfull[:])
                hTv = hTi[:].rearrange("p (n d) -> p n d", d=4)

                wh = p2.tile([128, 3, CW], bf16, tag="wh")
                nc.vector.memset(wh[:, :, 0:4], 0.0)
                slot = p2.tile([128, 3 * CW], f32, tag="slot")
                erg = p2.tile([128, EPW], f32, tag="erg")
                bnd = p2.tile([128, 400], f32, tag="bnd")
                negM = p2.tile([128, 1], f32, tag="negM")
                pe = pp2.tile([128, EPW], f32, tag="pe")

                ghr = slot[:, 0:2 * EPW].bitcast(bf16).rearrange(
                    "p (e d) -> p e d", d=4)               # [128, 4096, 4]
                e2v = slot[:, 2 * EPW:3 * EPW]             # [128, 4096] f32

                for w in range(NW):
                    hix = combs[:, w, 0:256]
                    eix = combs[:, w, 256:512]
                    bix = combs[:, w, 512:537]
                    nc.gpsimd.ap_gather(ghr, hTv, hix, channels=128,
                                        num_elems=ncores * SH, d=4,
                                        num_idxs=EPW)
                    nc.gpsimd.ap_gather(erg[:, :, None],
                                        er_rep[:, :, None], eix,
                                        channels=128, num_elems=SH, d=1,
                                        num_idxs=EPW)
                    # e = (el_hi + el_lo) + er
                    nc.vector.tensor_tensor(out=pe[:], in0=ghr[:, :, 2],
                                            in1=ghr[:, :, 3], op=add)
                    nc.vector.tensor_tensor(out=e2v, in0=pe[:], in1=erg[:],
                                            op=add)
                    # e = leakyrelu(e) = max(0.2*e, e)
                    nc.vector.scalar_tensor_tensor(
                        out=pe[:], in0=e2v, scalar=NEG_SLOPE, in1=e2v,
                        op0=mult, op1=amax)
                    # per-window max (negated) for a safe exp
                    nc.vector.tensor_reduce(
                        out=negM[:], in_=pe[:], axis=mybir.AxisListType.X,
                        op=amax, negate=True)
                    # w = exp(e - M) -> channel 2 of the scan buffer
                    nc.scalar.activation(
                        wh[:, 2, 4:4 + EPW], pe[:],
                        mybir.ActivationFunctionType.Exp,
                        bias=negM[:, 0:1])
                    # wh channels 0,1 = w * h-pair
                    nc.vector.tensor_tensor(
                        out=wh[:, 0:2, 4:4 + EPW].transpose([0, 2, 1]),
                        in0=ghr[:, :, 0:2],
                        in1=wh[:, 2, 4:4 + EPW][:, :, None].broadcast_to(
                            [128, EPW, 2]),
                        op=mult)
                    # one prefix scan over all 3 channels
                    nc.vector.tensor_tensor_scan(
                        out=slot[:],
                        data0=wh[:].rearrange("p a b -> p (a b)"),
                        data1=zrow[:].broadcast_to([128, 3 * CW]),
                        initial=0.0, op0=add, op1=add)
                    # segment boundary prefix values, then diff -> (u0,u1,s)
                    nc.gpsimd.ap_gather(bnd[:, :, None], slot[:, :, None],
                                        bix, channels=128,
                                        num_elems=3 * CW, d=1, num_idxs=400)
                    bv = bnd[:, 0:396].rearrange("p (k j) -> p k j", k=3)
                    nc.vector.tensor_tensor(
                        out=usall[:, :, w * 128:(w + 1) * 128],
                        in0=bv[:, :, 1:129], in1=bv[:, :, 0:128], op=sub)

            # ---- Epilogue: out^T = u / s + bias ----
            with tc.tile_pool(name="ep", bufs=1) as ep:
                rcp = ep.tile([128, SH], f32, tag="rcp")
                nc.vector.reciprocal(rcp[:], usall[:, 2, :])
                ot = ep.tile([128, 2, SH], f32, tag="ot")
                nc.vector.tensor_tensor(
                    out=ot[:], in0=usall[:, 0:2, :],
                    in1=rcp[:, None, :].broadcast_to([128, 2, SH]), op=mult)
                ot2 = ep.tile([128, 2, SH], f32, tag="ot2")
                nc.vector.tensor_tensor(
                    out=ot2[:], in0=ot[:],
                    in1=b2[:, :, None].broadcast_to([128, 2, SH]), op=add)
                nc.sync.dma_start(out_ext[:], ot2[:])

    nc.compile()
    return nc


def _get_program(ncores):
    if ncores not in _prog_cache:
        _prog_cache[ncores] = _build_program(ncores)
    return _prog_cache[ncores]


def kernel(feats, W, attn_l, attn_r, bias, src, dst):
    from concourse.bass_utils import run_bass_kernel_spmd

    feats = np.asarray(feats, np.float32)
    W = np.asarray(W, np.float32)
    attn_l = np.asarray(attn_l, np.float32)
    attn_r = np.asarray(attn_r, np.float32)
    bias = np.asarray(bias, np.float32)
    src = np.asarray(src)
    dst = np.asarray(dst)

    in_maps, newid = _prep_inputs(feats, W, attn_l, attn_r, bias, src, dst)
    nc = _get_program(NCORES)
    res = run_bass_kernel_spmd(nc, in_maps, list(range(NCORES)))
    shards = []
    for c in range(NCORES):
        o = np.asarray(res.results[c]["out"])          # [128, 2, SH]
        shards.append(o.transpose(2, 1, 0).reshape(SH, D))
    out_cat = np.concatenate(shards, axis=0)
    return out_cat[newid[:N]].astype(np.float32)


# revision 8
# speedup vs baseline: 5.5201x; 1.0291x over previous
"""Single-head GAT (DGL GATConv) forward on 8 Trainium2 NeuronCores.

Strategy (graph/data parallel, per the sharding hint), v2 — prefix-sum
aggregation. This environment executes instructions at a large flat cost
(~60-200us each, engine-dependent: DVE ~60us, Act ~110us, PE matmul
~190us, ap_gather [128,4096,4] ~120us, AllGather ~1ms), so the kernel is
shaped to minimize INSTRUCTION COUNT, using the largest possible access
patterns per instruction.

  - Nodes padded 10000 -> 10240, degree-balance relabeled into 80 windows
    of 128 dst nodes such that each window has <= 4096 in-edges; 10
    windows per core (1280 dst nodes/core); edges dst-sorted per window.
  - Phase 1 (per core): hT = W^T @ feats^T computed FEATURE-MAJOR: the
    local shard's table row n holds bf16 (h[n,p], h[n,p+128], el_hi,
    el_lo) on partition p (el split hi/lo bf16 for ~f32 accuracy);
    el/er rows from a [256,2] (W@attn_l | W@attn_r) matmul,
    partition-replicated via the gpsimd partition_broadcast ucode op.
    Staged to DRAM, AllGather -> full table [128, 10240, 4].
  - Phase 2 (per core, per window): ONE ap_gather pulls all 4096 edges'
    (h-pair, el-pair) columns; a second ap_gather pulls per-edge er (f32,
    by local dst). Logits e = lrelu(el+er) batched on [128,4096]; a
    per-window max is subtracted before exp (softmax shift-invariance per
    dst; all of a dst's edges live in one window, so it cancels exactly)
    which keeps the prefix sums O(1)-magnitude. wh[e] = w_e * h-pair and
    w_e are packed into a [128, 3, 4+4096] bf16 buffer (4 leading zeros
    per channel); ONE tensor_tensor_scan computes the running sum of all
    3 channels; ONE ap_gather reads the 3*129 segment-boundary prefix
    values; ONE subtract turns them into per-dst (u0, u1, s) =
    (sum w*h_p, sum w*h_{p+128}, sum w). Segment sums via prefix
    differences are exact in structure (the scan state is fp32).
  - Epilogue: out^T = u/s + bias in 3 DVE ops + 1 DMA; host de-transposes
    and un-relabels.

kernel(**inputs) takes full unsharded inputs, returns [10000, 256] fp32.
"""

import numpy as np
import ml_dtypes

N = 10000
E = 320000
D = 256
NPAD = 10240
NCORES = 8
SH = NPAD // NCORES          # 1280 nodes per core
WINN = 128                   # dst nodes per window
NW = SH // WINN              # 10 windows per core
EPW = 4096                   # padded edges per window
CW = 4100                    # scan channel width: 4 zero pads + 4096
NEG_SLOPE = 0.2
REPEAT = 1                   # whole-pipeline repeats (differential timing)

_BF16 = ml_dtypes.bfloat16

_prog_cache = {}


def _wrap16(arr):
    """[Q] -> [128, Q//16] int16 in ap_gather's wrapped-index layout:
    idx[16g + r, f] = arr[f*16 + r], replicated across the 8 groups g."""
    q = arr.shape[0]
    w16 = arr.reshape(q // 16, 16).T.astype(np.int16)     # [16, Q//16]
    return np.broadcast_to(w16[None], (8, 16, q // 16)).reshape(128, q // 16)


def _prep_inputs(feats, W, attn_l, attn_r, bias, src, dst):
    """Host-side sharding/index prep. Returns (in_maps, newid)."""
    feats_pad = np.zeros((NPAD, D), np.float32)
    feats_pad[:N] = feats
    val = (W @ attn_l).astype(np.float32)
    var = (W @ attn_r).astype(np.float32)

    # degree-balanced node relabeling: greedy-pack nodes into 80 windows of
    # 128 so every window has <= EPW in-edges
    import heapq
    nwin_g = NPAD // WINN                     # 80
    deg = np.bincount(dst, minlength=NPAD).astype(np.int64)
    norder = np.argsort(-deg, kind="stable")
    bin_edges = np.zeros(nwin_g, np.int64)
    bin_count = np.zeros(nwin_g, np.int64)
    newid = np.empty(NPAD, np.int64)
    heap = [(0, b) for b in range(nwin_g)]
    heapq.heapify(heap)
    for n in norder:
        while True:
            _, b = heapq.heappop(heap)
            if bin_count[b] < WINN:
                break
        newid[n] = b * WINN + bin_count[b]
        bin_count[b] += 1
        bin_edges[b] += deg[n]
        if bin_count[b] < WINN:
            heapq.heappush(heap, (bin_edges[b], b))
    assert bin_edges.max() <= EPW, (bin_edges.max(), EPW)
    inv = np.empty(NPAD, np.int64)
    inv[newid] = np.arange(NPAD)
    feats_pad = feats_pad[inv]                # row j of feats_pad = new id j

    n_src = newid[src.astype(np.int64)]
    n_dst = newid[dst.astype(np.int64)]
    order = np.argsort(n_dst, kind="stable")
    s_src = n_src[order]
    s_dst = n_dst[order]

    win = s_dst // WINN                       # global window id, 0..79
    counts = np.bincount(win, minlength=nwin_g)
    starts = np.concatenate([[0], np.cumsum(counts)])

    # per-window index tables: h-gather (global src), er-gather (local dst),
    # boundary-gather (3 channels x 129 prefix positions in the scan buffer)
    # 544 (not 537): keeps every idx slice 64-byte aligned in all
    # windows; odd-element offsets make the gpsimd idx reads garbage
    comb = np.zeros((nwin_g, 128, 544), np.int16)
    for g in range(nwin_g):
        a, b = starts[g], starts[g + 1]
        k = b - a
        hidx = np.zeros(EPW, np.int64)
        eidx = np.zeros(EPW, np.int64)
        hidx[:k] = s_src[a:b]
        core_base = (g // NW) * SH
        eidx[:k] = s_dst[a:b] - core_base
        slot = s_dst[a:b] - g * WINN          # 0..127, ascending
        cum = np.zeros(129, np.int64)         # cum[j] = #edges with slot < j
        cnt = np.bincount(slot, minlength=128)
        cum[1:] = np.cumsum(cnt)
        bidx = np.zeros(400, np.int64)
        for ch in range(3):
            bidx[132 * ch:132 * ch + 129] = CW * ch + 3 + cum
        comb[g, :, 0:256] = _wrap16(hidx)
        comb[g, :, 256:512] = _wrap16(eidx)
        comb[g, :, 512:537] = _wrap16(bidx)

    # weight tiles: Wt4[kp, kh, ft, fp] = W[kp+128kh, fp+128ft]
    w4 = W.astype(np.float32).reshape(2, 128, 2, 128)          # [kh,kp,ft,fp]
    Wt4 = np.ascontiguousarray(w4.transpose(1, 0, 2, 3))       # [kp,kh,ft,fp]
    valvar = np.stack([val, var], axis=1).reshape(2, 128, 2)   # [kh,kp,2]
    valvar = np.ascontiguousarray(valvar.transpose(1, 0, 2))   # [kp,kh,2]
    bias2 = np.ascontiguousarray(
        bias.astype(np.float32).reshape(2, 128).T)             # [p, d]

    in_maps = []
    for c in range(NCORES):
        fsh = feats_pad[c * SH:(c + 1) * SH]                   # [SH, 256]
        ftT = np.ascontiguousarray(
            fsh.T.reshape(2, 128, SH).transpose(1, 0, 2))      # [kp,kh,n]
        in_maps.append({
            "ftT": ftT,
            "Wt4": Wt4,
            "valvar": valvar,
            "bias2": bias2,
            "comb": np.ascontiguousarray(comb[c * NW:(c + 1) * NW]),
        })
    return in_maps, newid


def _build_program(ncores):
    import concourse.bass as bass
    import concourse.tile as tile
    from concourse import bacc, mybir
    from contextlib import ExitStack

    f32 = mybir.dt.float32
    bf16 = mybir.dt.bfloat16
    f16 = mybir.dt.float16
    i16 = mybir.dt.int16

    nc = bacc.Bacc(
        "TRN2", target_bir_lowering=False, debug=False, num_devices=ncores
    )

    ftT_in = nc.dram_tensor("ftT", [128, 2, SH], f32, kind="ExternalInput").ap()
    Wt4_in = nc.dram_tensor("Wt4", [128, 2, 2, 128], f32, kind="ExternalInput").ap()
    vv_in = nc.dram_tensor("valvar", [128, 2, 2], f32, kind="ExternalInput").ap()
    b2_in = nc.dram_tensor("bias2", [128, 2], f32, kind="ExternalInput").ap()
    comb_in = nc.dram_tensor("comb", [NW, 128, 544], i16, kind="ExternalInput").ap()
    out_ext = nc.dram_tensor("out", [128, 2, SH], f32, kind="ExternalOutput").ap()

    hstage = nc.dram_tensor("hstage", [128, SH * 4], bf16).ap()
    if ncores > 1:
        hfull = nc.dram_tensor("hfull", [ncores, 128, SH * 4], bf16,
                               addr_space="Shared").ap()
    else:
        hfull = hstage

    add = mybir.AluOpType.add
    mult = mybir.AluOpType.mult
    amax = mybir.AluOpType.max
    sub = mybir.AluOpType.subtract

    with tile.TileContext(nc) as tc, ExitStack() as ctx:
        const = ctx.enter_context(tc.tile_pool(name="const", bufs=1))
        wt = const.tile([128, 2, 2, 128], f32, tag="wt")
        nc.sync.dma_start(wt[:], Wt4_in[:])
        vv = const.tile([128, 2, 2], f32, tag="vv")
        nc.sync.dma_start(vv[:], vv_in[:])
        b2 = const.tile([128, 2], f32, tag="b2")
        nc.sync.dma_start(b2[:], b2_in[:])
        combs = const.tile([128, NW, 544], i16, tag="combs")
        nc.sync.dma_start(combs[:], comb_in.transpose([1, 0, 2]))
        zrow = const.tile([128, 1], f32, tag="zrow")
        nc.vector.memset(zrow[:], 0.0)
        usall = const.tile([128, 3, SH], f32, tag="usall")
        er_rep = const.tile([128, SH], f32, tag="er_rep")

        for _rep in range(REPEAT):
            # ---- Phase 1: feature-major h + el/er for the local shard ----
            with tc.tile_pool(name="p1", bufs=1) as p1:
                ftT = p1.tile([128, 2, SH], f32, tag="ftT")
                nc.sync.dma_start(ftT[:], ftT_in[:])

                bounds = [(0, 512), (512, 1024), (1024, 1280)]
                els = p1.tile([2, SH], f32, tag="els")
                with tc.tile_pool(name="p1pse", bufs=1, space="PSUM") as ppe:
                    pse = ppe.tile([2, 1536], f32, tag="pse")
                    for kh in range(2):
                        for c0, c1 in bounds:
                            nc.tensor.matmul(
                                pse[:, c0:c1], lhsT=vv[:, kh, :],
                                rhs=ftT[:, kh, c0:c1],
                                start=(kh == 0), stop=(kh == 1))
                    nc.vector.tensor_copy(els[:], pse[:, 0:SH])

                hst = p1.tile([128, SH, 4], bf16, tag="hst")
                with tc.tile_pool(name="p1psh", bufs=1, space="PSUM") as pph:
                    psh = pph.tile([128, 2, 1536], f32, tag="psh")
                    for ft in range(2):
                        for kh in range(2):
                            for c0, c1 in bounds:
                                nc.tensor.matmul(
                                    psh[:, ft, c0:c1], lhsT=wt[:, kh, ft, :],
                                    rhs=ftT[:, kh, c0:c1],
                                    start=(kh == 0), stop=(kh == 1))
                    nc.vector.tensor_copy(hst[:, :, 0], psh[:, 0, 0:SH])
                    nc.vector.tensor_copy(hst[:, :, 1], psh[:, 1, 0:SH])
                el_rep = p1.tile([128, SH], f32, tag="el_rep")
                ers = p1.tile([1, SH], f32, tag="ers")
                nc.sync.dma_start(ers[:], els[1:2, :])
                nc.gpsimd.partition_broadcast(el_rep[:], els[0:1, :],
                                              channels=128)
                nc.gpsimd.partition_broadcast(er_rep[:], ers[0:1, :],
                                              channels=128)
                # el as fp16 bits in the bf16 lane 2 (abs err ~6e-4);
                # lane 3 is a pad the gather fetches but nothing reads
                nc.vector.tensor_copy(hst[:, :, 2].bitcast(f16), el_rep[:])
                nc.sync.dma_start(
                    hstage[:], hst[:].rearrange("p n d -> p (n d)"))

            if ncores > 1:
                nc.gpsimd.collective_compute(
                    "AllGather", mybir.AluOpType.bypass,
                    replica_groups=[list(range(ncores))],
                    ins=[hstage[:]], outs=[hfull[:]],
                )

            # ---- Phase 2: per-window gather + softmax + prefix-sum agg ----
            with tc.tile_pool(name="p2", bufs=1) as p2, \
                 tc.tile_pool(name="p2ps", bufs=1, space="PSUM") as pp2:
                hTi = p2.tile([128, ncores * SH * 4], bf16, tag="hTi")
                if ncores > 1:
                    nc.sync.dma_start(hTi[:], hfull.transpose([1, 0, 2]))
                else:
                    nc.sync.dma_start(hTi[:], hfull[:])
                hTv = hTi[:].rearrange("p (n d) -> p n d", d=4)

                wh = p2.tile([128, 3, CW], bf16, tag="wh")
                nc.vector.memset(wh[:, :, 0:4], 0.0)
                slot = p2.tile([128, 3 * CW], f32, tag="slot")
                erg = p2.tile([128, EPW], f32, tag="erg")
                bnd = p2.tile([128, 400], f32, tag="bnd")
                negM = p2.tile([128, 1], f32, tag="negM")
                pe = pp2.tile([128, EPW], f32, tag="pe")

                ghr = slot[:, 0:2 * EPW].bitcast(bf16).rearrange(
                    "p (e d) -> p e d", d=4)               # [128, 4096, 4]
                e2v = slot[:, 2 * EPW:3 * EPW]             # [128, 4096] f32

                for w in range(NW):
                    hix = combs[:, w, 0:256]
                    eix = combs[:, w, 256:512]
                    bix = combs[:, w, 512:537]
                    nc.gpsimd.ap_gather(ghr, hTv, hix, channels=128,
                                        num_elems=ncores * SH, d=4,
                                        num_idxs=EPW)
                    nc.gpsimd.ap_gather(erg[:, :, None],
                                        er_rep[:, :, None], eix,
                                        channels=128, num_elems=SH, d=1,
                                        num_idxs=EPW)
                    # e = el + er  (el was stored as fp16 bits)
                    nc.vector.tensor_tensor(out=e2v,
                                            in0=ghr[:, :, 2].bitcast(f16),
                                            in1=erg[:], op=add)
                    # e = leakyrelu(e) = max(0.2*e, e)
                    nc.vector.scalar_tensor_tensor(
                        out=pe[:], in0=e2v, scalar=NEG_SLOPE, in1=e2v,
                        op0=mult, op1=amax)
                    # per-window max (negated) for a safe exp
                    nc.vector.tensor_reduce(
                        out=negM[:], in_=pe[:], axis=mybir.AxisListType.X,
                        op=amax, negate=True)
                    # w = exp(e - M) -> channel 2 of the scan buffer
                    nc.scalar.activation(
                        wh[:, 2, 4:4 + EPW], pe[:],
                        mybir.ActivationFunctionType.Exp,
                        bias=negM[:, 0:1])
                    # wh channels 0,1 = w * h-pair
                    nc.vector.tensor_tensor(
                        out=wh[:, 0:2, 4:4 + EPW].transpose([0, 2, 1]),
                        in0=ghr[:, :, 0:2],
                        in1=wh[:, 2, 4:4 + EPW][:, :, None].broadcast_to(
                            [128, EPW, 2]),
                        op=mult)
                    # one prefix scan over all 3 channels
                    nc.vector.tensor_tensor_scan(
                        out=slot[:],
                        data0=wh[:].rearrange("p a b -> p (a b)"),
                        data1=zrow[:].broadcast_to([128, 3 * CW]),
                        initial=0.0, op0=add, op1=add)
                    # segment boundary prefix values, then diff -> (u0,u1,s)
                    nc.gpsimd.ap_gather(bnd[:, :, None], slot[:, :, None],
                                        bix, channels=128,
                                        num_elems=3 * CW, d=1, num_idxs=400)
                    bv = bnd[:, 0:396].rearrange("p (k j) -> p k j", k=3)
                    nc.vector.tensor_tensor(
                        out=usall[:, :, w * 128:(w + 1) * 128],
                        in0=bv[:, :, 1:129], in1=bv[:, :, 0:128], op=sub)

            # ---- Epilogue: out^T = u / s + bias ----
            with tc.tile_pool(name="ep", bufs=1) as ep:
                rcp = ep.tile([128, SH], f32, tag="rcp")
                nc.vector.reciprocal(rcp[:], usall[:, 2, :])
                ot = ep.tile([128, 2, SH], f32, tag="ot")
                nc.vector.tensor_tensor(
                    out=ot[:], in0=usall[:, 0:2, :],
                    in1=rcp[:, None, :].broadcast_to([128, 2, SH]), op=mult)
                ot2 = ep.tile([128, 2, SH], f32, tag="ot2")
                nc.vector.tensor_tensor(
                    out=ot2[:], in0=ot[:],
                    in1=b2[:, :, None].broadcast_to([128, 2, SH]), op=add)
                nc.sync.dma_start(out_ext[:], ot2[:])

    nc.compile()
    return nc


def _get_program(ncores):
    if ncores not in _prog_cache:
        _prog_cache[ncores] = _build_program(ncores)
    return _prog_cache[ncores]


def kernel(feats, W, attn_l, attn_r, bias, src, dst):
    from concourse.bass_utils import run_bass_kernel_spmd

    feats = np.asarray(feats, np.float32)
    W = np.asarray(W, np.float32)
    attn_l = np.asarray(attn_l, np.float32)
    attn_r = np.asarray(attn_r, np.float32)
    bias = np.asarray(bias, np.float32)
    src = np.asarray(src)
    dst = np.asarray(dst)

    in_maps, newid = _prep_inputs(feats, W, attn_l, attn_r, bias, src, dst)
    nc = _get_program(NCORES)
    res = run_bass_kernel_spmd(nc, in_maps, list(range(NCORES)))
    shards = []
    for c in range(NCORES):
        o = np.asarray(res.results[c]["out"])          # [128, 2, SH]
        shards.append(o.transpose(2, 1, 0).reshape(SH, D))
    out_cat = np.concatenate(shards, axis=0)
    return out_cat[newid[:N]].astype(np.float32)


# revision 9
# speedup vs baseline: 7.5554x; 1.3687x over previous
"""Single-head GAT (DGL GATConv) forward on 8 Trainium2 NeuronCores.

Strategy (graph/data parallel, per the sharding hint), v2 — prefix-sum
aggregation. This environment executes instructions at a large flat cost
(~60-200us each, engine-dependent: DVE ~60us, Act ~110us, PE matmul
~190us, ap_gather [128,4096,4] ~120us, AllGather ~1ms), so the kernel is
shaped to minimize INSTRUCTION COUNT, using the largest possible access
patterns per instruction.

  - Nodes padded 10000 -> 10240, degree-balance relabeled into 80 windows
    of 128 dst nodes such that each window has <= 4096 in-edges; 10
    windows per core (1280 dst nodes/core); edges dst-sorted per window.
  - Phase 1 (per core): hT = W^T @ feats^T computed FEATURE-MAJOR: the
    local shard's table row n holds bf16 (h[n,p], h[n,p+128], el_hi,
    el_lo) on partition p (el split hi/lo bf16 for ~f32 accuracy);
    el/er rows from a [256,2] (W@attn_l | W@attn_r) matmul,
    partition-replicated via the gpsimd partition_broadcast ucode op.
    Staged to DRAM, AllGather -> full table [128, 10240, 4].
  - Phase 2 (per core, per window): ONE ap_gather pulls all 4096 edges'
    (h-pair, el-pair) columns; a second ap_gather pulls per-edge er (f32,
    by local dst). Logits e = lrelu(el+er) batched on [128,4096]; a
    per-window max is subtracted before exp (softmax shift-invariance per
    dst; all of a dst's edges live in one window, so it cancels exactly)
    which keeps the prefix sums O(1)-magnitude. wh[e] = w_e * h-pair and
    w_e are packed into a [128, 3, 4+4096] bf16 buffer (4 leading zeros
    per channel); ONE tensor_tensor_scan computes the running sum of all
    3 channels; ONE ap_gather reads the 3*129 segment-boundary prefix
    values; ONE subtract turns them into per-dst (u0, u1, s) =
    (sum w*h_p, sum w*h_{p+128}, sum w). Segment sums via prefix
    differences are exact in structure (the scan state is fp32).
  - Epilogue: out^T = u/s + bias in 3 DVE ops + 1 DMA; host de-transposes
    and un-relabels.

kernel(**inputs) takes full unsharded inputs, returns [10000, 256] fp32.
"""

import numpy as np
import ml_dtypes

N = 10000
E = 320000
D = 256
NPAD = 10240
NCORES = 8
SH = NPAD // NCORES          # 1280 nodes per core
WINN = 128                   # dst nodes per window
NW = SH // WINN              # 10 windows per core
EPW = 4096                   # padded edges per window
CW = 4100                    # scan channel width: 4 zero pads + 4096
NEG_SLOPE = 0.2
REPEAT = 1                   # whole-pipeline repeats (differential timing)

_BF16 = ml_dtypes.bfloat16

_prog_cache = {}


def _wrap16(arr):
    """[Q] -> [128, Q//16] int16 in ap_gather's wrapped-index layout:
    idx[16g + r, f] = arr[f*16 + r], replicated across the 8 groups g."""
    q = arr.shape[0]
    w16 = arr.reshape(q // 16, 16).T.astype(np.int16)     # [16, Q//16]
    return np.broadcast_to(w16[None], (8, 16, q // 16)).reshape(128, q // 16)


def _prep_inputs(feats, W, attn_l, attn_r, bias, src, dst):
    """Host-side sharding/index prep. Returns (in_maps, newid)."""
    feats_pad = np.zeros((NPAD, D), np.float32)
    feats_pad[:N] = feats

    # degree-balanced node relabeling: greedy-pack nodes into 80 windows of
    # 128 so every window has <= EPW in-edges
    import heapq
    nwin_g = NPAD // WINN                     # 80
    deg = np.bincount(dst, minlength=NPAD).astype(np.int64)
    norder = np.argsort(-deg, kind="stable")
    bin_edges = np.zeros(nwin_g, np.int64)
    bin_count = np.zeros(nwin_g, np.int64)
    newid = np.empty(NPAD, np.int64)
    heap = [(0, b) for b in range(nwin_g)]
    heapq.heapify(heap)
    for n in norder:
        while True:
            _, b = heapq.heappop(heap)
            if bin_count[b] < WINN:
                break
        newid[n] = b * WINN + bin_count[b]
        bin_count[b] += 1
        bin_edges[b] += deg[n]
        if bin_count[b] < WINN:
            heapq.heappush(heap, (bin_edges[b], b))
    assert bin_edges.max() <= EPW, (bin_edges.max(), EPW)
    inv = np.empty(NPAD, np.int64)
    inv[newid] = np.arange(NPAD)
    feats_pad = feats_pad[inv]                # row j of feats_pad = new id j

    n_src = newid[src.astype(np.int64)]
    n_dst = newid[dst.astype(np.int64)]
    order = np.argsort(n_dst, kind="stable")
    s_src = n_src[order]
    s_dst = n_dst[order]

    win = s_dst // WINN                       # global window id, 0..79
    counts = np.bincount(win, minlength=nwin_g)
    starts = np.concatenate([[0], np.cumsum(counts)])

    # per-window index tables: h-gather (global src), er-gather (local dst),
    # boundary-gather (3 channels x 129 prefix positions in the scan buffer)
    # 544 (not 537): keeps every idx slice 64-byte aligned in all
    # windows; odd-element offsets make the gpsimd idx reads garbage
    comb = np.zeros((nwin_g, 128, 544), np.int16)
    for g in range(nwin_g):
        a, b = starts[g], starts[g + 1]
        k = b - a
        hidx = np.zeros(EPW, np.int64)
        eidx = np.zeros(EPW, np.int64)
        hidx[:k] = s_src[a:b]
        core_base = (g // NW) * SH
        eidx[:k] = s_dst[a:b] - core_base
        slot = s_dst[a:b] - g * WINN          # 0..127, ascending
        cum = np.zeros(129, np.int64)         # cum[j] = #edges with slot < j
        cnt = np.bincount(slot, minlength=128)
        cum[1:] = np.cumsum(cnt)
        bidx = np.zeros(400, np.int64)
        for ch in range(3):
            bidx[132 * ch:132 * ch + 129] = CW * ch + 3 + cum
        comb[g, :, 0:256] = _wrap16(hidx)
        comb[g, :, 256:512] = _wrap16(eidx)
        comb[g, :, 512:537] = _wrap16(bidx)

    # weight tiles: Wt4[kp, kh, ft, fp] = W[kp+128kh, fp+128ft]
    w4 = W.astype(np.float32).reshape(2, 128, 2, 128)          # [kh,kp,ft,fp]
    Wt4 = np.ascontiguousarray(w4.transpose(1, 0, 2, 3))       # [kp,kh,ft,fp]
    # attn4[p, lr, ft] = (attn_l | attn_r)[p + 128*ft]
    attn4 = np.stack([attn_l.astype(np.float32).reshape(2, 128).T,
                      attn_r.astype(np.float32).reshape(2, 128).T],
                     axis=1)                                   # [p, lr, ft]
    bias2 = np.ascontiguousarray(
        bias.astype(np.float32).reshape(2, 128).T)             # [p, d]

    in_maps = []
    for c in range(NCORES):
        fsh = feats_pad[c * SH:(c + 1) * SH]                   # [SH, 256]
        ftT = np.ascontiguousarray(
            fsh.T.reshape(2, 128, SH).transpose(1, 0, 2))      # [kp,kh,n]
        in_maps.append({
            "ftT": ftT,
            "Wt4": Wt4,
            "attn4": attn4,
            "bias2": bias2,
            "comb": np.ascontiguousarray(comb[c * NW:(c + 1) * NW]),
        })
    return in_maps, newid


def _build_program(ncores):
    import concourse.bass as bass
    import concourse.tile as tile
    from concourse import bacc, mybir
    from contextlib import ExitStack

    f32 = mybir.dt.float32
    bf16 = mybir.dt.bfloat16
    f16 = mybir.dt.float16
    i16 = mybir.dt.int16

    nc = bacc.Bacc(
        "TRN2", target_bir_lowering=False, debug=False, num_devices=ncores
    )

    ftT_in = nc.dram_tensor("ftT", [128, 2, SH], f32, kind="ExternalInput").ap()
    Wt4_in = nc.dram_tensor("Wt4", [128, 2, 2, 128], f32, kind="ExternalInput").ap()
    a4_in = nc.dram_tensor("attn4", [128, 2, 2], f32, kind="ExternalInput").ap()
    b2_in = nc.dram_tensor("bias2", [128, 2], f32, kind="ExternalInput").ap()
    comb_in = nc.dram_tensor("comb", [NW, 128, 544], i16, kind="ExternalInput").ap()
    out_ext = nc.dram_tensor("out", [128, 2, SH], f32, kind="ExternalOutput").ap()

    hstage = nc.dram_tensor("hstage", [128, SH * 4], bf16).ap()
    if ncores > 1:
        hfull = nc.dram_tensor("hfull", [ncores, 128, SH * 4], bf16,
                               addr_space="Shared").ap()
    else:
        hfull = hstage

    add = mybir.AluOpType.add
    mult = mybir.AluOpType.mult
    amax = mybir.AluOpType.max
    sub = mybir.AluOpType.subtract

    with tile.TileContext(nc) as tc, ExitStack() as ctx:
        const = ctx.enter_context(tc.tile_pool(name="const", bufs=1))
        wt = const.tile([128, 2, 2, 128], f32, tag="wt")
        nc.sync.dma_start(wt[:], Wt4_in[:])
        a4 = const.tile([128, 2, 2], f32, tag="a4")
        nc.sync.dma_start(a4[:], a4_in[:])
        b2 = const.tile([128, 2], f32, tag="b2")
        nc.sync.dma_start(b2[:], b2_in[:])
        combs = const.tile([128, NW, 544], i16, tag="combs")
        nc.sync.dma_start(combs[:], comb_in.transpose([1, 0, 2]))
        zrow = const.tile([128, 1], f32, tag="zrow")
        nc.vector.memset(zrow[:], 0.0)
        usall = const.tile([128, 3, SH], f32, tag="usall")
        er_rep = const.tile([128, SH], f32, tag="er_rep")

        for _rep in range(REPEAT):
            # ---- Phase 1: feature-major h + el/er for the local shard ----
            with tc.tile_pool(name="p1", bufs=1) as p1:
                ftT = p1.tile([128, 2, SH], f32, tag="ftT")
                nc.sync.dma_start(ftT[:], ftT_in[:])

                bounds = [(0, 512), (512, 1024), (1024, 1280)]
                hst = p1.tile([128, SH, 4], bf16, tag="hst")
                t4 = p1.tile([128, 2, 2, SH], f32, tag="t4")
                with tc.tile_pool(name="p1psh", bufs=1, space="PSUM") as pph:
                    psh = pph.tile([128, 2, 1536], f32, tag="psh")
                    for ft in range(2):
                        for kh in range(2):
                            for c0, c1 in bounds:
                                nc.tensor.matmul(
                                    psh[:, ft, c0:c1], lhsT=wt[:, kh, ft, :],
                                    rhs=ftT[:, kh, c0:c1],
                                    start=(kh == 0), stop=(kh == 1))
                    nc.vector.tensor_copy(hst[:, :, 0], psh[:, 0, 0:SH])
                    nc.vector.tensor_copy(hst[:, :, 1], psh[:, 1, 0:SH])
                    # t4[p, lr, ft, n] = attn4[p, lr, ft] * hT[p, ft, n];
                    # partition-sum + lane-pair add = el/er, replicated
                    nc.vector.tensor_tensor(
                        out=t4[:],
                        in0=psh[:, None, :, 0:SH].broadcast_to([128, 2, 2, SH]),
                        in1=a4[:, :, :, None].broadcast_to([128, 2, 2, SH]),
                        op=mult)
                import concourse.bass_isa as bass_isa
                t4r = p1.tile([128, 2, 2, SH], f32, tag="t4r")
                nc.gpsimd.partition_all_reduce(
                    t4r[:].rearrange("p a b n -> p (a b n)"),
                    t4[:].rearrange("p a b n -> p (a b n)"),
                    channels=128, reduce_op=bass_isa.ReduceOp.add)
                el_rep = p1.tile([128, SH], f32, tag="el_rep")
                nc.vector.tensor_tensor(out=el_rep[:], in0=t4r[:, 0, 0, :],
                                        in1=t4r[:, 0, 1, :], op=add)
                nc.vector.tensor_tensor(out=er_rep[:], in0=t4r[:, 1, 0, :],
                                        in1=t4r[:, 1, 1, :], op=add)
                # el as fp16 bits in the bf16 lane 2 (abs err ~6e-4);
                # lane 3 is a pad the gather fetches but nothing reads
                nc.vector.tensor_copy(hst[:, :, 2].bitcast(f16), el_rep[:])
                nc.sync.dma_start(
                    hstage[:], hst[:].rearrange("p n d -> p (n d)"))

            if ncores > 1:
                nc.gpsimd.collective_compute(
                    "AllGather", mybir.AluOpType.bypass,
                    replica_groups=[list(range(ncores))],
                    ins=[hstage[:]], outs=[hfull[:]],
                )

            # ---- Phase 2: per-window gather + softmax + prefix-sum agg ----
            with tc.tile_pool(name="p2", bufs=1) as p2, \
                 tc.tile_pool(name="p2ps", bufs=1, space="PSUM") as pp2:
                hTi = p2.tile([128, ncores * SH * 4], bf16, tag="hTi")
                if ncores > 1:
                    nc.sync.dma_start(hTi[:], hfull.transpose([1, 0, 2]))
                else:
                    nc.sync.dma_start(hTi[:], hfull[:])
                hTv = hTi[:].rearrange("p (n d) -> p n d", d=4)

                wh = p2.tile([128, 3, CW], bf16, tag="wh")
                nc.vector.memset(wh[:, :, 0:4], 0.0)
                slot = p2.tile([128, 3 * CW], f32, tag="slot")
                erg = p2.tile([128, EPW], f32, tag="erg")
                bnd = p2.tile([128, 400], f32, tag="bnd")
                negM = p2.tile([128, 1], f32, tag="negM")
                pe = pp2.tile([128, EPW], f32, tag="pe")

                ghr = slot[:, 0:2 * EPW].bitcast(bf16).rearrange(
                    "p (e d) -> p e d", d=4)               # [128, 4096, 4]
                e2v = slot[:, 2 * EPW:3 * EPW]             # [128, 4096] f32

                for w in range(NW):
                    hix = combs[:, w, 0:256]
                    eix = combs[:, w, 256:512]
                    bix = combs[:, w, 512:537]
                    nc.gpsimd.ap_gather(ghr, hTv, hix, channels=128,
                                        num_elems=ncores * SH, d=4,
                                        num_idxs=EPW)
                    nc.gpsimd.ap_gather(erg[:, :, None],
                                        er_rep[:, :, None], eix,
                                        channels=128, num_elems=SH, d=1,
                                        num_idxs=EPW)
                    # e = el + er  (el was stored as fp16 bits)
                    nc.vector.tensor_tensor(out=e2v,
                                            in0=ghr[:, :, 2].bitcast(f16),
                                            in1=erg[:], op=add)
                    # e = leakyrelu(e) = max(0.2*e, e)
                    nc.vector.scalar_tensor_tensor(
                        out=pe[:], in0=e2v, scalar=NEG_SLOPE, in1=e2v,
                        op0=mult, op1=amax)
                    # per-window max (negated) for a safe exp
                    nc.vector.tensor_reduce(
                        out=negM[:], in_=pe[:], axis=mybir.AxisListType.X,
                        op=amax, negate=True)
                    # w = exp(e - M) -> channel 2 of the scan buffer
                    nc.scalar.activation(
                        wh[:, 2, 4:4 + EPW], pe[:],
                        mybir.ActivationFunctionType.Exp,
                        bias=negM[:, 0:1])
                    # wh channels 0,1 = w * h-pair
                    nc.vector.tensor_tensor(
                        out=wh[:, 0:2, 4:4 + EPW].transpose([0, 2, 1]),
                        in0=ghr[:, :, 0:2],
                        in1=wh[:, 2, 4:4 + EPW][:, :, None].broadcast_to(
                            [128, EPW, 2]),
                        op=mult)
                    # one prefix scan over all 3 channels
                    nc.vector.tensor_tensor_scan(
                        out=slot[:],
                        data0=wh[:].rearrange("p a b -> p (a b)"),
                        data1=zrow[:].broadcast_to([128, 3 * CW]),
                        initial=0.0, op0=add, op1=add)
                    # segment boundary prefix values, then diff -> (u0,u1,s)
                    nc.gpsimd.ap_gather(bnd[:, :, None], slot[:, :, None],
                                        bix, channels=128,
                                        num_elems=3 * CW, d=1, num_idxs=400)
                    bv = bnd[:, 0:396].rearrange("p (k j) -> p k j", k=3)
                    nc.vector.tensor_tensor(
                        out=usall[:, :, w * 128:(w + 1) * 128],
                        in0=bv[:, :, 1:129], in1=bv[:, :, 0:128], op=sub)

            # ---- Epilogue: out^T = u / s + bias ----
            with tc.tile_pool(name="ep", bufs=1) as ep:
                rcp = ep.tile([128, SH], f32, tag="rcp")
                nc.vector.reciprocal(rcp[:], usall[:, 2, :])
                ot = ep.tile([128, 2, SH], f32, tag="ot")
                nc.vector.tensor_tensor(
                    out=ot[:], in0=usall[:, 0:2, :],
                    in1=rcp[:, None, :].broadcast_to([128, 2, SH]), op=mult)
                ot2 = ep.tile([128, 2, SH], f32, tag="ot2")
                nc.vector.tensor_tensor(
                    out=ot2[:], in0=ot[:],
                    in1=b2[:, :, None].broadcast_to([128, 2, SH]), op=add)
                nc.sync.dma_start(out_ext[:], ot2[:])

    nc.compile()
    return nc


def _get_program(ncores):
    if ncores not in _prog_cache:
        _prog_cache[ncores] = _build_program(ncores)
    return _prog_cache[ncores]


def kernel(feats, W, attn_l, attn_r, bias, src, dst):
    from concourse.bass_utils import run_bass_kernel_spmd

    feats = np.asarray(feats, np.float32)
    W = np.asarray(W, np.float32)
    attn_l = np.asarray(attn_l, np.float32)
    attn_r = np.asarray(attn_r, np.float32)
    bias = np.asarray(bias, np.float32)
    src = np.asarray(src)
    dst = np.asarray(dst)

    in_maps, newid = _prep_inputs(feats, W, attn_l, attn_r, bias, src, dst)
    nc = _get_program(NCORES)
    res = run_bass_kernel_spmd(nc, in_maps, list(range(NCORES)))
    shards = []
    for c in range(NCORES):
        o = np.asarray(res.results[c]["out"])          # [128, 2, SH]
        shards.append(o.transpose(2, 1, 0).reshape(SH, D))
    out_cat = np.concatenate(shards, axis=0)
    return out_cat[newid[:N]].astype(np.float32)
